# revision 5
# baseline (speedup 1.0000x reference)
"""Causal self-attention (B=2, T=2048, C=1024, nh=16) on 8 TRN2 NeuronCores.

Sharding: core c = 4*b + g handles batch b (2048 tokens) and head-group g
(4 heads).  Megatron-style: QKV rows and proj columns sharded by head group;
the proj partial sums are reduced on the host (the "all-reduce").

Per-core kernel (all matmuls in float32r = full-rate tf32-class):
  1. QKV projection  kqvT[f,t] = Wl @ x_b.T   (f = [k4|q4|v4] x 64, q pre-scaled)
  2. v tiles transposed on PE to [s,d] layout, packed with shared
     [ones|zeros] columns so the PV matmul also emits softmax row-sums
  3. per head: S.T = kT.T@qT per (s-tile, t-chunk) causally, additive mask on
     diagonal blocks, exp on ACT, PV accumulation v_aug.T @ P.T -> yT + rowsum
  4. normalize via DVE recip + PE ones-broadcast, proj partial on PE
"""

import os
import numpy as np

B, T, C, NH, HD = 2, 2048, 1024, 16, 64
HPC = 4  # heads per core
NCORES = 8
NEG = -1.0e30

_cache = {}


def _build_nc():
    from contextlib import ExitStack

    import concourse.bass as bass
    import concourse.tile as tile
    from concourse import bacc, mybir

    f32 = mybir.dt.float32
    f32r = mybir.dt.float32r
    AF = mybir.ActivationFunctionType
    OP = mybir.AluOpType

    nc = bacc.Bacc("TRN2", target_bir_lowering=False, debug=False,
                   num_devices=NCORES)

    xt = nc.dram_tensor("xt", [C, T], f32r, kind="ExternalInput").ap()
    wkqv = nc.dram_tensor("wkqv", [C, 3 * HPC * HD], f32r,
                          kind="ExternalInput").ap()
    bkq = nc.dram_tensor("bkq", [128, 6], f32, kind="ExternalInput").ap()
    wproj = nc.dram_tensor("wproj", [HPC * HD, C], f32r,
                           kind="ExternalInput").ap()
    bp = nc.dram_tensor("bp", [128, 8], f32, kind="ExternalInput").ap()
    ident_d = nc.dram_tensor("ident", [128, 128], f32,
                             kind="ExternalInput").ap()
    amask_d = nc.dram_tensor("amask", [128, 128], f32,
                             kind="ExternalInput").ap()
    vmid_d = nc.dram_tensor("vmid", [128, 4096], f32r,
                            kind="ExternalInput").ap()
    ones_d = nc.dram_tensor("ones64", [128, 64], f32r,
                            kind="ExternalInput").ap()
    outp = nc.dram_tensor("outp", [C, T], f32, kind="ExternalOutput").ap()

    NM = 6        # kqv m-tiles
    NCH = 4       # 512-wide t-chunks
    NJ = 16       # 128-wide s-tiles
    CHW = 512

    with tile.TileContext(nc) as tc, ExitStack() as ctx:
        sing = ctx.enter_context(tc.tile_pool(name="sing", bufs=1))
        xpool = ctx.enter_context(tc.tile_pool(name="xpool", bufs=2))
        ptp = ctx.enter_context(tc.tile_pool(name="ptp", bufs=3))
        rsp = ctx.enter_context(tc.tile_pool(name="rsp", bufs=2))
        rbp = ctx.enter_context(tc.tile_pool(name="rbp", bufs=2))
        osp = ctx.enter_context(tc.tile_pool(name="osp", bufs=3))
        ps = ctx.enter_context(tc.tile_pool(name="ps", bufs=2, space="PSUM"))

        # ---- resident SBUF tensors ----
        wk = sing.tile([128, 8, NM * 128], f32r, name="wk")
        kqv = sing.tile([128, NM, T], f32r, name="kqv")
        # vsb: 32 blocks of [v_A(64) | SH1(64) | SH2(64) | v_B(64)]
        vsb = sing.tile([128, 32 * 256], f32r, name="vsb")
        ysb = sing.tile([128, 2, T], f32r, name="ysb")
        wp = sing.tile([128, 2, C], f32r, name="wp")
        bkq_s = sing.tile([128, 6], f32, name="bkq_s")
        bp_s = sing.tile([128, 8], f32, name="bp_s")
        ident = sing.tile([128, 128], f32, name="ident")
        amask = sing.tile([128, 128], f32, name="amask")
        ones_r = sing.tile([128, 64], f32r, name="ones_r")

        nc.sync.dma_start(wk, wkqv.rearrange("(kk p) f -> p kk f", p=128))
        nc.sync.dma_start(wp, wproj.rearrange("(kk p) f -> p kk f", p=128))
        nc.sync.dma_start(bkq_s, bkq)
        nc.sync.dma_start(bp_s, bp)
        nc.sync.dma_start(ident, ident_d)
        nc.sync.dma_start(amask, amask_d)
        vanchor = vsb[:, 64:65]
        vmid_view = bass.AP(tensor=vanchor.tensor, offset=vanchor.offset,
                            ap=[vanchor.ap[0], [256, 32], [1, 128]])
        nc.sync.dma_start(vmid_view,
                          vmid_d.rearrange("p (a b) -> p a b", a=32))
        nc.sync.dma_start(ones_r, ones_d)

        xt_r = xt.rearrange("(kk p) t -> p kk t", p=128)

        # ---- phase 1: QKV projection ----
        with nc.named_scope("qkv"):
            for n in range(NCH):
                xts = xpool.tile([128, 8, CHW], f32r, name="xts")
                nc.sync.dma_start(xts, xt_r[:, :, n * CHW:(n + 1) * CHW])
                for m in range(NM):
                    acc = ps.tile([128, CHW], f32, name="acc", tag="acc")
                    for k in range(8):
                        nc.tensor.matmul(
                            acc, wk[:, k, m * 128:(m + 1) * 128], xts[:, k, :],
                            start=(k == 0), stop=(k == 7))
                    nc.vector.tensor_scalar_add(
                        out=kqv[:, m, n * CHW:(n + 1) * CHW], in0=acc,
                        scalar1=bkq_s[:, m:m + 1])

        # ---- phase 2: v transposes into vsb ----
        with nc.named_scope("vprep"):
            for j in range(NJ):
                for hf in range(2):
                    pt_ps = ps.tile([128, 128], f32, name="pt_ps", tag="aux")
                    nc.tensor.transpose(
                        pt_ps,
                        kqv[:, 4 + hf, j * 128:(j + 1) * 128].bitcast(f32),
                        ident)
                    off = (j * 2 + hf) * 256
                    anch = vsb[:, off:off + 1]
                    dst = bass.AP(tensor=anch.tensor, offset=anch.offset,
                                  ap=[anch.ap[0], [192, 2], [1, 64]])
                    nc.vector.tensor_copy(
                        dst, pt_ps.rearrange("p (a b) -> p a b", a=2))

        def v_stationary(j, h):
            """[128,128] AP: even slot -> [v_A|SH1], odd -> [SH2|v_B]."""
            hf, sl = h // 2, h % 2
            off = (j * 2 + hf) * 256 + 128 * sl
            return vsb[:, off:off + 128]

        # ---- phase 3: attention per head ----
        with nc.named_scope("attn"):
            for h in range(HPC):
                hf, sl = h // 2, h % 2
                p0 = 64 * sl
                kT = kqv[p0:p0 + 64, hf, :]
                qT = kqv[p0:p0 + 64, 2 + hf, :]
                rp = 64 if sl == 0 else 0  # rowsum partition in py
                for n in range(NCH):
                    jmax = 4 * n + 3
                    py = ps.tile([128, CHW], f32, name="py", tag="py")
                    pend = None  # deferred PV emission for pipelining
                    for j in range(jmax + 1):
                        c0 = max(0, 128 * j - 512 * n)
                        w = CHW - c0
                        ss = ps.tile([128, CHW], f32, name="ss", tag="ss")
                        nc.tensor.matmul(
                            ss[:, c0:], kT[:, j * 128:(j + 1) * 128],
                            qT[:, n * CHW + c0:(n + 1) * CHW],
                            start=True, stop=True)
                        if pend is not None:
                            pend()
                            pend = None
                        if j >= 4 * n:  # diagonal block: additive causal mask
                            nc.vector.tensor_tensor(
                                out=ss[:, c0:c0 + 128], in0=ss[:, c0:c0 + 128],
                                in1=amask, op=mybir.AluOpType.add)
                        pt = ptp.tile([128, CHW], f32r, name="pt")
                        nc.scalar.activation(out=pt[:, c0:], in_=ss[:, c0:],
                                             func=AF.Exp)

                        def mk_pv(j=j, c0=c0, pt=pt):
                            nc.tensor.matmul(
                                py[:, c0:], v_stationary(j, h), pt[:, c0:],
                                start=(j == 0), stop=(j == jmax),
                                skip_group_check=True)
                        pend = mk_pv
                    pend()
                    # normalize: recip rowsum, PE broadcast, copy, multiply
                    rs = rsp.tile([128, CHW], f32r, name="rs")
                    with nc.allow_low_precision(reason="f32r recip ok"):
                        nc.vector.reciprocal(rs[rp:rp + 1, :],
                                             py[rp:rp + 1, :])
                    pb = ps.tile([128, CHW], f32, name="pb", tag="aux")
                    nc.tensor.matmul(
                        pb[0:64, :], ones_r[rp:rp + 1, :],
                        rs[rp:rp + 1, :], start=True, stop=True)
                    rb = rbp.tile([128, CHW], f32, name="rb")
                    nc.vector.tensor_copy(rb[0:64, :], pb[0:64, :])
                    ob = 64 * sl
                    if sl == 1:
                        # matmul cannot target psum base 64; shift via DMA
                        nc.sync.dma_start(rb[64:128, :], rb[0:64, :])
                    nc.vector.tensor_tensor(
                        out=ysb[ob:ob + 64, hf, n * CHW:(n + 1) * CHW],
                        in0=py[ob:ob + 64, :], in1=rb[ob:ob + 64, :],
                        op=OP.mult)

        # ---- phase 4: proj partial ----
        with nc.named_scope("proj"):
            for o in range(8):
                for n in range(NCH):
                    acc = ps.tile([128, CHW], f32, name="acc2", tag="acc")
                    for kk in range(2):
                        nc.tensor.matmul(
                            acc, wp[:, kk, o * 128:(o + 1) * 128],
                            ysb[:, kk, n * CHW:(n + 1) * CHW],
                            start=(kk == 0), stop=(kk == 1))
                    ot = osp.tile([128, CHW], f32, name="ot")
                    nc.vector.tensor_scalar_add(out=ot, in0=acc,
                                                scalar1=bp_s[:, o:o + 1])
                    nc.sync.dma_start(
                        outp[o * 128:(o + 1) * 128, n * CHW:(n + 1) * CHW], ot)

    nc.compile()
    return nc


def _host_inputs(x, W_kqv, b_kqv, W_proj, b_proj):
    x = np.ascontiguousarray(np.asarray(x, dtype=np.float32))
    W_kqv = np.asarray(W_kqv, dtype=np.float32)
    b_kqv = np.asarray(b_kqv, dtype=np.float32)
    W_proj = np.asarray(W_proj, dtype=np.float32)
    b_proj = np.asarray(b_proj, dtype=np.float32)

    ident = np.eye(128, dtype=np.float32)
    ss, tt = np.meshgrid(np.arange(128), np.arange(128), indexing="ij")
    amask = np.where(ss <= tt, 0.0, NEG).astype(np.float32)
    vmid1 = np.zeros((128, 128), dtype=np.float32)
    vmid1[:, 0] = 1.0    # SH1 col 0: rowsum at out partition 64 (even slot)
    vmid1[:, 64] = 1.0   # SH2 col 0: rowsum at out partition 0 (odd slot)
    vmid = np.tile(vmid1, (1, 32))
    ones64 = np.ones((128, 64), dtype=np.float32)

    xts = [np.ascontiguousarray(x[b].T) for b in range(B)]

    in_maps = []
    for c in range(NCORES):
        b, g = c // 4, c % 4
        heads = [4 * g + i for i in range(HPC)]
        wl = np.concatenate(
            [W_kqv[h * 192:h * 192 + 64] for h in heads]
            + [W_kqv[h * 192 + 64:h * 192 + 128] * 0.125 for h in heads]
            + [W_kqv[h * 192 + 128:h * 192 + 192] for h in heads], axis=0)
        bl = np.concatenate(
            [b_kqv[h * 192:h * 192 + 64] for h in heads]
            + [b_kqv[h * 192 + 64:h * 192 + 128] * 0.125 for h in heads]
            + [b_kqv[h * 192 + 128:h * 192 + 192] for h in heads])
        bpl = b_proj if g == 0 else np.zeros_like(b_proj)
        in_maps.append({
            "xt": xts[b],
            "wkqv": np.ascontiguousarray(wl.T),
            "bkq": np.ascontiguousarray(bl.reshape(6, 128).T),
            "wproj": np.ascontiguousarray(
                W_proj[:, 256 * g:256 * (g + 1)].T),
            "bp": np.ascontiguousarray(bpl.reshape(8, 128).T),
            "ident": ident,
            "amask": amask,
            "vmid": vmid,
            "ones64": ones64,
        })
    return in_maps


def kernel(x, W_kqv, b_kqv, W_proj, b_proj):
    from concourse.bass_utils import run_bass_kernel_spmd

    if "nc" not in _cache:
        _cache["nc"] = _build_nc()
    nc = _cache["nc"]

    in_maps = _host_inputs(x, W_kqv, b_kqv, W_proj, b_proj)
    trace = bool(int(os.environ.get("KERNEL_TRACE", "0")))
    r = run_bass_kernel_spmd(nc, in_maps, core_ids=list(range(NCORES)),
                             trace=trace)
    if trace:
        _cache["last_results"] = r
        print(f"HW exec time: {r.exec_time_ns} ns")

    out = np.empty((B, T, C), dtype=np.float32)
    for b in range(B):
        acc = np.zeros((C, T), dtype=np.float32)
        for g in range(4):
            acc += r.results[4 * b + g]["outp"]
        out[b] = acc.T
    return out


# revision 7
# speedup vs baseline: 1.1112x; 1.1112x over previous
"""Causal self-attention (B=2, T=2048, C=1024, nh=16) on 8 TRN2 NeuronCores.

Sharding: core c = 4*b + g handles batch b (2048 tokens) and head-group g
(4 heads).  Megatron-style: QKV rows and proj columns sharded by head group;
the proj partial sums are reduced on the host (the "all-reduce").

Per-core kernel (all matmuls in float32r = full-rate tf32-class):
  1. QKV projection  kqvT[f,t] = Wl @ x_b.T   (f = [k4|q4|v4] x 64, q pre-scaled)
  2. v tiles transposed on PE to [s,d] layout, packed with shared
     [ones|zeros] columns so the PV matmul also emits softmax row-sums
  3. per head: S.T = kT.T@qT per (s-tile, t-chunk) causally, additive mask on
     diagonal blocks, exp on ACT, PV accumulation v_aug.T @ P.T -> yT + rowsum
  4. normalize via DVE recip + PE ones-broadcast, proj partial on PE
"""

import os
import numpy as np

B, T, C, NH, HD = 2, 2048, 1024, 16, 64
HPC = 4  # heads per core
NCORES = 8
NEG = -1.0e30

_cache = {}


def _build_nc():
    from contextlib import ExitStack

    import concourse.bass as bass
    import concourse.tile as tile
    from concourse import bacc, mybir

    f32 = mybir.dt.float32
    f32r = mybir.dt.float32r
    AF = mybir.ActivationFunctionType
    OP = mybir.AluOpType

    nc = bacc.Bacc("TRN2", target_bir_lowering=False, debug=False,
                   num_devices=NCORES)

    xt = nc.dram_tensor("xt", [C, T], f32r, kind="ExternalInput").ap()
    wkqv = nc.dram_tensor("wkqv", [C, 3 * HPC * HD], f32r,
                          kind="ExternalInput").ap()
    bkq = nc.dram_tensor("bkq", [128, 6], f32, kind="ExternalInput").ap()
    wproj = nc.dram_tensor("wproj", [HPC * HD, C], f32r,
                           kind="ExternalInput").ap()
    bp = nc.dram_tensor("bp", [128, 8], f32, kind="ExternalInput").ap()
    ident_d = nc.dram_tensor("ident", [128, 128], f32,
                             kind="ExternalInput").ap()
    amask_d = nc.dram_tensor("amask", [128, 128], f32,
                             kind="ExternalInput").ap()
    vmid_d = nc.dram_tensor("vmid", [128, 4096], f32r,
                            kind="ExternalInput").ap()
    outp = nc.dram_tensor("outp", [C, T], f32, kind="ExternalOutput").ap()

    NM = 6        # kqv m-tiles
    NCH = 4       # 512-wide t-chunks
    NJ = 16       # 128-wide s-tiles
    CHW = 512

    with tile.TileContext(nc) as tc, ExitStack() as ctx:
        sing = ctx.enter_context(tc.tile_pool(name="sing", bufs=1))
        xpool = ctx.enter_context(tc.tile_pool(name="xpool", bufs=2))
        ptp = ctx.enter_context(tc.tile_pool(name="ptp", bufs=3))
        rsp = ctx.enter_context(tc.tile_pool(name="rsp", bufs=2))
        rbp = ctx.enter_context(tc.tile_pool(name="rbp", bufs=2))
        osp = ctx.enter_context(tc.tile_pool(name="osp", bufs=3))
        ps = ctx.enter_context(tc.tile_pool(name="ps", bufs=2, space="PSUM"))

        # ---- resident SBUF tensors ----
        wk = sing.tile([128, 8, NM * 128], f32r, name="wk")
        kqv = sing.tile([128, NM, T], f32r, name="kqv")
        # vsb: 32 blocks of [v_A(64) | SH1(64) | SH2(64) | v_B(64)]
        vsb = sing.tile([128, 32 * 256], f32r, name="vsb")
        ysb = sing.tile([128, 2, T], f32r, name="ysb")
        wp = sing.tile([128, 2, C], f32r, name="wp")
        bkq_s = sing.tile([128, 6], f32, name="bkq_s")
        bp_s = sing.tile([128, 8], f32, name="bp_s")
        ident = sing.tile([128, 128], f32, name="ident")
        amask = sing.tile([128, 128], f32, name="amask")

        wkqv_r = wkqv.rearrange("(kk p) f -> p kk f", p=128)
        for k in range(8):
            nc.sync.dma_start(wk[:, k, :], wkqv_r[:, k, :])
        nc.sync.dma_start(wp, wproj.rearrange("(kk p) f -> p kk f", p=128))
        nc.sync.dma_start(bkq_s, bkq)
        nc.sync.dma_start(bp_s, bp)
        nc.sync.dma_start(ident, ident_d)
        nc.sync.dma_start(amask, amask_d)
        vanchor = vsb[:, 64:65]
        vmid_view = bass.AP(tensor=vanchor.tensor, offset=vanchor.offset,
                            ap=[vanchor.ap[0], [256, 32], [1, 128]])
        nc.sync.dma_start(vmid_view,
                          vmid_d.rearrange("p (a b) -> p a b", a=32))

        xt_r = xt.rearrange("(kk p) t -> p kk t", p=128)

        # ---- phase 1: QKV projection ----
        with nc.named_scope("qkv"):
            for n in range(NCH):
                xts = xpool.tile([128, 8, CHW], f32r, name="xts")
                nc.sync.dma_start(xts, xt_r[:, :, n * CHW:(n + 1) * CHW])
                for m in range(NM):
                    acc = ps.tile([128, CHW], f32, name="acc", tag="acc")
                    for k in range(8):
                        nc.tensor.matmul(
                            acc, wk[:, k, m * 128:(m + 1) * 128], xts[:, k, :],
                            start=(k == 0), stop=(k == 7))
                    nc.vector.tensor_scalar_add(
                        out=kqv[:, m, n * CHW:(n + 1) * CHW], in0=acc,
                        scalar1=bkq_s[:, m:m + 1])

        # ---- phase 2: v transposes into vsb ----
        with nc.named_scope("vprep"):
            for j in range(NJ):
                for hf in range(2):
                    pt_ps = ps.tile([128, 128], f32, name="pt_ps", tag="aux")
                    nc.tensor.transpose(
                        pt_ps,
                        kqv[:, 4 + hf, j * 128:(j + 1) * 128].bitcast(f32),
                        ident)
                    off = (j * 2 + hf) * 256
                    anch = vsb[:, off:off + 1]
                    dst = bass.AP(tensor=anch.tensor, offset=anch.offset,
                                  ap=[anch.ap[0], [192, 2], [1, 64]])
                    nc.vector.tensor_copy(
                        dst, pt_ps.rearrange("p (a b) -> p a b", a=2))

        def v_stationary(j, h):
            """[128,128] AP: even slot -> [v_A|SH1], odd -> [SH2|v_B]."""
            hf, sl = h // 2, h % 2
            off = (j * 2 + hf) * 256 + 128 * sl
            return vsb[:, off:off + 128]

        # ---- phase 3: attention per head ----
        with nc.named_scope("attn"):
            for h in range(HPC):
                hf, sl = h // 2, h % 2
                p0 = 64 * sl
                kT = kqv[p0:p0 + 64, hf, :]
                qT = kqv[p0:p0 + 64, 2 + hf, :]
                for n in range(NCH):
                    jmax = 4 * n + 3
                    py = ps.tile([128, CHW], f32, name="py", tag="acc")
                    pend = None  # deferred PV emission for pipelining
                    for j in range(jmax + 1):
                        c0 = max(0, 128 * j - 512 * n)
                        w = CHW - c0
                        ss = ps.tile([128, CHW], f32, name="ss", tag="ss",
                                     bufs=4)
                        nc.tensor.matmul(
                            ss[:, c0:], kT[:, j * 128:(j + 1) * 128],
                            qT[:, n * CHW + c0:(n + 1) * CHW],
                            start=True, stop=True)
                        if pend is not None:
                            pend()
                            pend = None
                        if j >= 4 * n:  # diagonal block: additive causal mask
                            nc.vector.tensor_tensor(
                                out=ss[:, c0:c0 + 128], in0=ss[:, c0:c0 + 128],
                                in1=amask, op=mybir.AluOpType.add)
                        pt = ptp.tile([128, CHW], f32r, name="pt")
                        nc.scalar.activation(out=pt[:, c0:], in_=ss[:, c0:],
                                             func=AF.Exp)

                        def mk_pv(j=j, c0=c0, pt=pt):
                            nc.tensor.matmul(
                                py[:, c0:], v_stationary(j, h), pt[:, c0:],
                                start=(j == 0), stop=(j == jmax),
                                skip_group_check=True)
                        pend = mk_pv
                    pend()
                    # normalize: rowsum arrives pre-broadcast on the
                    # complement partitions; recip there, DMA-shift, multiply
                    ob = 64 * sl       # yT partitions
                    cb = 64 - ob       # rowsum-broadcast partitions
                    rbt = rsp.tile([128, CHW], f32, name="rbt")
                    nc.vector.reciprocal(rbt[cb:cb + 64, :],
                                         py[cb:cb + 64, :])
                    rb = rbp.tile([128, CHW], f32, name="rb")
                    nc.sync.dma_start(rb[ob:ob + 64, :], rbt[cb:cb + 64, :])
                    nc.vector.tensor_tensor(
                        out=ysb[ob:ob + 64, hf, n * CHW:(n + 1) * CHW],
                        in0=py[ob:ob + 64, :], in1=rb[ob:ob + 64, :],
                        op=OP.mult)

        # ---- phase 4: proj partial ----
        with nc.named_scope("proj"):
            for o in range(8):
                for n in range(NCH):
                    acc = ps.tile([128, CHW], f32, name="acc2", tag="acc")
                    for kk in range(2):
                        nc.tensor.matmul(
                            acc, wp[:, kk, o * 128:(o + 1) * 128],
                            ysb[:, kk, n * CHW:(n + 1) * CHW],
                            start=(kk == 0), stop=(kk == 1))
                    ot = osp.tile([128, CHW], f32, name="ot")
                    nc.scalar.activation(out=ot, in_=acc, func=AF.Identity,
                                         bias=bp_s[:, o:o + 1])
                    nc.sync.dma_start(
                        outp[o * 128:(o + 1) * 128, n * CHW:(n + 1) * CHW], ot)

    nc.compile()
    return nc


def _host_inputs(x, W_kqv, b_kqv, W_proj, b_proj):
    x = np.ascontiguousarray(np.asarray(x, dtype=np.float32))
    W_kqv = np.asarray(W_kqv, dtype=np.float32)
    b_kqv = np.asarray(b_kqv, dtype=np.float32)
    W_proj = np.asarray(W_proj, dtype=np.float32)
    b_proj = np.asarray(b_proj, dtype=np.float32)

    ident = np.eye(128, dtype=np.float32)
    ss, tt = np.meshgrid(np.arange(128), np.arange(128), indexing="ij")
    amask = np.where(ss <= tt, 0.0, NEG).astype(np.float32)
    # all-ones mid columns: PV matmul emits rowsum broadcast over the
    # complement 64 partitions of each head's yT
    vmid = np.ones((128, 4096), dtype=np.float32)

    xts = [np.ascontiguousarray(x[b].T) for b in range(B)]

    in_maps = []
    for c in range(NCORES):
        b, g = c // 4, c % 4
        heads = [4 * g + i for i in range(HPC)]
        wl = np.concatenate(
            [W_kqv[h * 192:h * 192 + 64] for h in heads]
            + [W_kqv[h * 192 + 64:h * 192 + 128] * 0.125 for h in heads]
            + [W_kqv[h * 192 + 128:h * 192 + 192] for h in heads], axis=0)
        bl = np.concatenate(
            [b_kqv[h * 192:h * 192 + 64] for h in heads]
            + [b_kqv[h * 192 + 64:h * 192 + 128] * 0.125 for h in heads]
            + [b_kqv[h * 192 + 128:h * 192 + 192] for h in heads])
        bpl = b_proj if g == 0 else np.zeros_like(b_proj)
        in_maps.append({
            "xt": xts[b],
            "wkqv": np.ascontiguousarray(wl.T),
            "bkq": np.ascontiguousarray(bl.reshape(6, 128).T),
            "wproj": np.ascontiguousarray(
                W_proj[:, 256 * g:256 * (g + 1)].T),
            "bp": np.ascontiguousarray(bpl.reshape(8, 128).T),
            "ident": ident,
            "amask": amask,
            "vmid": vmid,
        })
    return in_maps


def kernel(x, W_kqv, b_kqv, W_proj, b_proj):
    from concourse.bass_utils import run_bass_kernel_spmd

    if "nc" not in _cache:
        _cache["nc"] = _build_nc()
    nc = _cache["nc"]

    in_maps = _host_inputs(x, W_kqv, b_kqv, W_proj, b_proj)
    trace = bool(int(os.environ.get("KERNEL_TRACE", "0")))
    r = run_bass_kernel_spmd(nc, in_maps, core_ids=list(range(NCORES)),
                             trace=trace)
    if trace:
        _cache["last_results"] = r
        print(f"HW exec time: {r.exec_time_ns} ns")

    out = np.empty((B, T, C), dtype=np.float32)
    for b in range(B):
        acc = np.zeros((C, T), dtype=np.float32)
        for g in range(4):
            acc += r.results[4 * b + g]["outp"]
        out[b] = acc.T
    return out


# revision 11
# speedup vs baseline: 1.3643x; 1.2277x over previous
"""Causal self-attention (B=2, T=2048, C=1024, nh=16) on 8 TRN2 NeuronCores.

Sharding: core c = 4*b + g handles batch b (2048 tokens) and head-group g
(4 heads).  Megatron-style: QKV rows and proj columns sharded by head group;
the proj partial sums are reduced on the host (the "all-reduce").

Per-core kernel (all matmuls in float32r = full-rate tf32-class):
  1. QKV projection kqvT[f,t] = Wl @ x_b.T. k-outputs land in zero-padded
     per-slot buffers (ktp_e/ktp_o) so QK matmuls contract K=128 with a full
     128-partition moving operand (half-partition rhs streams at half rate).
  2. v tiles transposed on PE to [s,d] layout, packed next to shared all-ones
     column blocks so the PV matmul also emits softmax row-sums pre-broadcast
     across the complement 64 partitions.
  3. per head, per 512-token chunk: S.T = kT.T@qT per s-tile pair into 2-bank
     psum, additive causal mask on diagonal blocks, exp on ACT (merged over
     pairs when possible), PV accumulation v_aug.T @ P.T -> yT + rowsum.
  4. normalize via fast-approx reciprocal + DMA partition shift + multiply,
     proj partial on PE, biases folded into the PSUM->SBUF copies.
"""

import os
import numpy as np

B, T, C, NH, HD = 2, 2048, 1024, 16, 64
HPC = 4  # heads per core
NCORES = 8
NEG = -1.0e30

_cache = {}


def _build_nc():
    from contextlib import ExitStack

    import concourse.bass as bass
    import concourse.tile as tile
    from concourse import bacc, mybir

    f32 = mybir.dt.float32
    f32r = mybir.dt.float32r
    AF = mybir.ActivationFunctionType
    OP = mybir.AluOpType

    nc = bacc.Bacc("TRN2", target_bir_lowering=False, debug=False,
                   num_devices=NCORES)

    xt = nc.dram_tensor("xt", [C, T], f32r, kind="ExternalInput").ap()
    wkqv = nc.dram_tensor("wkqv", [C, 3 * HPC * HD], f32r,
                          kind="ExternalInput").ap()
    bkq = nc.dram_tensor("bkq", [128, 6], f32, kind="ExternalInput").ap()
    wproj = nc.dram_tensor("wproj", [HPC * HD, C], f32r,
                           kind="ExternalInput").ap()
    bp = nc.dram_tensor("bp", [128, 8], f32, kind="ExternalInput").ap()
    ident_d = nc.dram_tensor("ident", [128, 128], f32,
                             kind="ExternalInput").ap()
    amask_d = nc.dram_tensor("amask", [128, 128], f32,
                             kind="ExternalInput").ap()
    vmid_d = nc.dram_tensor("vmid", [128, 4096], f32r,
                            kind="ExternalInput").ap()
    outp = nc.dram_tensor("outp", [C, T], f32, kind="ExternalOutput").ap()

    NCH = 4       # 512-wide t-chunks
    NJ = 16       # 128-wide s-tiles
    CHW = 512

    with tile.TileContext(nc) as tc, ExitStack() as ctx:
        sing = ctx.enter_context(tc.tile_pool(name="sing", bufs=1))
        xpool = ctx.enter_context(tc.tile_pool(name="xpool", bufs=2))
        ptp = ctx.enter_context(tc.tile_pool(name="ptp", bufs=3))
        rsp = ctx.enter_context(tc.tile_pool(name="rsp", bufs=2))
        rbp = ctx.enter_context(tc.tile_pool(name="rbp", bufs=2))
        osp = ctx.enter_context(tc.tile_pool(name="osp", bufs=3))
        ps = ctx.enter_context(tc.tile_pool(name="ps", bufs=2, space="PSUM"))

        # ---- resident SBUF tensors ----
        wk = sing.tile([128, 8, 768], f32r, name="wk")
        # q (slots 0,1) and vT (slots 2,3) by head-pair
        kqv = sing.tile([128, 4, T], f32r, name="kqv")
        # zero-padded kT for full-K QK matmuls: even heads in rows 0:64 of
        # ktp_e (rows 64:128 zero), odd heads in rows 64:128 of ktp_o
        ktp_e = sing.tile([128, 2, T], f32r, name="ktp_e")
        ktp_o = sing.tile([128, 2, T], f32r, name="ktp_o")
        # vsb: 32 blocks of [v_A(64) | ones(128) | v_B(64)]
        vsb = sing.tile([128, 32 * 256], f32r, name="vsb")
        ysb = sing.tile([128, 2, T], f32r, name="ysb")
        wp = sing.tile([128, 2, C], f32r, name="wp")
        bkq_s = sing.tile([128, 6], f32, name="bkq_s")
        bp_s = sing.tile([128, 8], f32, name="bp_s")
        ident = sing.tile([128, 128], f32, name="ident")
        amask = sing.tile([128, 128], f32, name="amask")

        nc.vector.memset(ktp_e[64:128, :, :].bitcast(f32), 0.0)
        nc.vector.memset(ktp_o[0:64, :, :].bitcast(f32), 0.0)

        xt_r = xt.rearrange("(kk p) t -> p kk t", p=128)
        wkqv_r = wkqv.rearrange("(kk p) f -> p kk f", p=128)
        for k in range(8):
            nc.sync.dma_start(wk[:, k, :], wkqv_r[:, k, :])
        nc.sync.dma_start(wp, wproj.rearrange("(kk p) f -> p kk f", p=128))
        nc.sync.dma_start(bkq_s, bkq)
        nc.sync.dma_start(bp_s, bp)
        nc.sync.dma_start(ident, ident_d)
        nc.sync.dma_start(amask, amask_d)
        vanchor = vsb[:, 64:65]
        vmid_view = bass.AP(tensor=vanchor.tensor, offset=vanchor.offset,
                            ap=[vanchor.ap[0], [256, 32], [1, 128]])
        nc.sync.dma_start(vmid_view,
                          vmid_d.rearrange("p (a b) -> p a b", a=32))

        # ---- phase 1: QKV projection ----
        # m: 0,1 -> kT (head pairs), 2,3 -> q, 4,5 -> v
        with nc.named_scope("qkv"):
            for n in range(NCH):
                xts = xpool.tile([128, 8, CHW], f32r, name="xts")
                for k in range(8):
                    nc.sync.dma_start(
                        xts[:, k, :],
                        xt_r[:, k, n * CHW:(n + 1) * CHW])
                for m in range(6):
                    acc = ps.tile([128, CHW], f32, name="acc", tag="acc")
                    for k in range(8):
                        nc.tensor.matmul(
                            acc, wk[:, k, m * 128:(m + 1) * 128], xts[:, k, :],
                            start=(k == 0), stop=(k == 7))
                    cols = slice(n * CHW, (n + 1) * CHW)
                    if m < 2:  # kT: split into zero-padded buffers
                        nc.vector.tensor_scalar_add(
                            out=ktp_e[0:64, m, cols], in0=acc[0:64, :],
                            scalar1=bkq_s[0:64, m:m + 1])
                        nc.vector.tensor_scalar_add(
                            out=ktp_o[64:128, m, cols], in0=acc[64:128, :],
                            scalar1=bkq_s[64:128, m:m + 1])
                    else:
                        nc.vector.tensor_scalar_add(
                            out=kqv[:, m - 2, cols], in0=acc,
                            scalar1=bkq_s[:, m:m + 1])

        # ---- phase 2: v transposes into vsb ----
        with nc.named_scope("vprep"):
            for j in range(NJ):
                for hf in range(2):
                    pt_ps = ps.tile([128, 128], f32, name="pt_ps", tag="aux")
                    nc.tensor.transpose(
                        pt_ps,
                        kqv[:, 2 + hf, j * 128:(j + 1) * 128].bitcast(f32),
                        ident)
                    off = (j * 2 + hf) * 256
                    anch = vsb[:, off:off + 1]
                    dst = bass.AP(tensor=anch.tensor, offset=anch.offset,
                                  ap=[anch.ap[0], [192, 2], [1, 64]])
                    nc.vector.tensor_copy(
                        dst, pt_ps.rearrange("p (a b) -> p a b", a=2))

        def v_stationary(j, h):
            """[128,128] AP: even slot -> [v_A|ones64], odd -> [ones64|v_B]."""
            hf, sl = h // 2, h % 2
            off = (j * 2 + hf) * 256 + 128 * sl
            return vsb[:, off:off + 128]

        # ---- phase 3: attention per head ----
        with nc.named_scope("attn"):
            for h in range(HPC):
                hf, sl = h // 2, h % 2
                ktp = ktp_e if sl == 0 else ktp_o
                for n in range(NCH):
                    jmax = 4 * n + 3
                    py = ps.tile([128, CHW], f32, name="py", tag="acc")
                    pend = []
                    for a in range(2 * n + 2):
                        ss = ps.tile([128, 2, CHW], f32, name="ss", tag="ss")
                        c0s = []
                        for idx in range(2):
                            j = 2 * a + idx
                            c0 = max(0, 128 * j - 512 * n)
                            c0s.append(c0)
                            nc.tensor.matmul(
                                ss[:, idx, c0:],
                                ktp[:, hf, j * 128:(j + 1) * 128],
                                kqv[:, hf, n * CHW + c0:(n + 1) * CHW],
                                start=True, stop=True)
                        for fn in pend:
                            fn()
                        pend = []
                        for idx in range(2):
                            j = 2 * a + idx
                            c0 = c0s[idx]
                            if j >= 4 * n:  # diagonal: additive causal mask
                                nc.vector.tensor_tensor(
                                    out=ss[:, idx, c0:c0 + 128],
                                    in0=ss[:, idx, c0:c0 + 128],
                                    in1=amask, op=OP.add)
                        pt = ptp.tile([128, 2, CHW], f32r, name="pt")
                        for idx in range(2):
                            c0 = c0s[idx]
                            nc.scalar.activation(
                                out=pt[:, idx, c0:], in_=ss[:, idx, c0:],
                                func=AF.Exp)

                        def mk_pv(a=a, c0s=c0s, pt=pt):
                            for idx in range(2):
                                j = 2 * a + idx
                                c0 = c0s[idx]
                                nc.tensor.matmul(
                                    py[:, c0:], v_stationary(j, h),
                                    pt[:, idx, c0:],
                                    start=(j == 0), stop=(j == jmax),
                                    skip_group_check=True)
                        pend = [mk_pv]
                    for fn in pend:
                        fn()
                    # normalize: rowsum arrives pre-broadcast on the
                    # complement partitions; recip there, DMA-shift, multiply
                    ob = 64 * sl       # yT partitions
                    cb = 64 - ob       # rowsum-broadcast partitions
                    rbt = rsp.tile([128, CHW], f32, name="rbt")
                    nc.vector.reciprocal(rbt[cb:cb + 64, :],
                                         py[cb:cb + 64, :])
                    rb = rbp.tile([128, CHW], f32, name="rb")
                    nc.sync.dma_start(rb[ob:ob + 64, :], rbt[cb:cb + 64, :])
                    nc.vector.tensor_tensor(
                        out=ysb[ob:ob + 64, hf, n * CHW:(n + 1) * CHW],
                        in0=py[ob:ob + 64, :], in1=rb[ob:ob + 64, :],
                        op=OP.mult)

        # ---- phase 4: proj partial ----
        with nc.named_scope("proj"):
            for o in range(8):
                for n in range(NCH):
                    acc = ps.tile([128, CHW], f32, name="acc2", tag="acc")
                    for kk in range(2):
                        nc.tensor.matmul(
                            acc, wp[:, kk, o * 128:(o + 1) * 128],
                            ysb[:, kk, n * CHW:(n + 1) * CHW],
                            start=(kk == 0), stop=(kk == 1))
                    ot = osp.tile([128, CHW], f32, name="ot")
                    nc.scalar.activation(out=ot, in_=acc, func=AF.Identity,
                                         bias=bp_s[:, o:o + 1])
                    nc.sync.dma_start(
                        outp[o * 128:(o + 1) * 128, n * CHW:(n + 1) * CHW], ot)

    nc.compile()
    return nc


def _host_inputs(x, W_kqv, b_kqv, W_proj, b_proj):
    x = np.ascontiguousarray(np.asarray(x, dtype=np.float32))
    W_kqv = np.asarray(W_kqv, dtype=np.float32)
    b_kqv = np.asarray(b_kqv, dtype=np.float32)
    W_proj = np.asarray(W_proj, dtype=np.float32)
    b_proj = np.asarray(b_proj, dtype=np.float32)

    ident = np.eye(128, dtype=np.float32)
    ss, tt = np.meshgrid(np.arange(128), np.arange(128), indexing="ij")
    amask = np.where(ss <= tt, 0.0, NEG).astype(np.float32)
    # all-ones mid columns: PV matmul emits rowsum broadcast over the
    # complement 64 partitions of each head's yT
    vmid = np.ones((128, 4096), dtype=np.float32)

    xts = [np.ascontiguousarray(x[b].T) for b in range(B)]

    in_maps = []
    for c in range(NCORES):
        b, g = c // 4, c % 4
        heads = [4 * g + i for i in range(HPC)]
        wl = np.concatenate(
            [W_kqv[h * 192:h * 192 + 64] for h in heads]
            + [W_kqv[h * 192 + 64:h * 192 + 128] * 0.125 for h in heads]
            + [W_kqv[h * 192 + 128:h * 192 + 192] for h in heads], axis=0)
        bl = np.concatenate(
            [b_kqv[h * 192:h * 192 + 64] for h in heads]
            + [b_kqv[h * 192 + 64:h * 192 + 128] * 0.125 for h in heads]
            + [b_kqv[h * 192 + 128:h * 192 + 192] for h in heads])
        bpl = b_proj if g == 0 else np.zeros_like(b_proj)
        in_maps.append({
            "xt": xts[b],
            "wkqv": np.ascontiguousarray(wl.T),
            "bkq": np.ascontiguousarray(bl.reshape(6, 128).T),
            "wproj": np.ascontiguousarray(
                W_proj[:, 256 * g:256 * (g + 1)].T),
            "bp": np.ascontiguousarray(bpl.reshape(8, 128).T),
            "ident": ident,
            "amask": amask,
            "vmid": vmid,
        })
    return in_maps


def kernel(x, W_kqv, b_kqv, W_proj, b_proj):
    from concourse.bass_utils import run_bass_kernel_spmd

    if "nc" not in _cache:
        _cache["nc"] = _build_nc()
    nc = _cache["nc"]

    in_maps = _host_inputs(x, W_kqv, b_kqv, W_proj, b_proj)
    trace = bool(int(os.environ.get("KERNEL_TRACE", "0")))
    r = run_bass_kernel_spmd(nc, in_maps, core_ids=list(range(NCORES)),
                             trace=trace)
    if trace:
        _cache["last_results"] = r
        print(f"HW exec time: {r.exec_time_ns} ns")

    out = np.empty((B, T, C), dtype=np.float32)
    for b in range(B):
        acc = np.zeros((C, T), dtype=np.float32)
        for g in range(4):
            acc += r.results[4 * b + g]["outp"]
        out[b] = acc.T
    return out


# revision 13
# speedup vs baseline: 1.3852x; 1.0153x over previous
"""Causal self-attention (B=2, T=2048, C=1024, nh=16) on 8 TRN2 NeuronCores.

Sharding: core c = 4*b + g handles batch b (2048 tokens) and head-group g
(4 heads).  Megatron-style: QKV rows and proj columns sharded by head group;
the proj partial sums are reduced on the host (the "all-reduce").

Per-core kernel (all matmuls in float32r = full-rate tf32-class):
  1. QKV projection kqvT[f,t] = Wl @ x_b.T. k-outputs land in zero-padded
     per-slot buffers (ktp_e/ktp_o) so QK matmuls contract K=128 with a full
     128-partition moving operand (half-partition rhs streams at half rate).
  2. v tiles transposed on PE to [s,d] layout, packed next to shared all-ones
     column blocks so the PV matmul also emits softmax row-sums pre-broadcast
     across the complement 64 partitions.
  3. per head, per 512-token chunk: S.T = kT.T@qT per s-tile pair into 2-bank
     psum, additive causal mask on diagonal blocks, exp on ACT (merged over
     pairs when possible), PV accumulation v_aug.T @ P.T -> yT + rowsum.
  4. normalize via fast-approx reciprocal + DMA partition shift + multiply,
     proj partial on PE, biases folded into the PSUM->SBUF copies.
"""

import os
import numpy as np

B, T, C, NH, HD = 2, 2048, 1024, 16, 64
HPC = 4  # heads per core
NCORES = 8
NEG = -1.0e30

_cache = {}


def _build_nc():
    from contextlib import ExitStack

    import concourse.bass as bass
    import concourse.tile as tile
    from concourse import bacc, mybir

    f32 = mybir.dt.float32
    f32r = mybir.dt.float32r
    AF = mybir.ActivationFunctionType
    OP = mybir.AluOpType

    nc = bacc.Bacc("TRN2", target_bir_lowering=False, debug=False,
                   num_devices=NCORES)

    xt = nc.dram_tensor("xt", [C, T], f32r, kind="ExternalInput").ap()
    wkqv = nc.dram_tensor("wkqv", [C, 3 * HPC * HD], f32r,
                          kind="ExternalInput").ap()
    bkq = nc.dram_tensor("bkq", [128, 6], f32, kind="ExternalInput").ap()
    wproj = nc.dram_tensor("wproj", [HPC * HD, C], f32r,
                           kind="ExternalInput").ap()
    bp = nc.dram_tensor("bp", [128, 8], f32, kind="ExternalInput").ap()
    ident_d = nc.dram_tensor("ident", [128, 128], f32,
                             kind="ExternalInput").ap()
    amask_d = nc.dram_tensor("amask", [128, 128], f32,
                             kind="ExternalInput").ap()
    vmid_d = nc.dram_tensor("vmid", [128, 4096], f32r,
                            kind="ExternalInput").ap()
    outp = nc.dram_tensor("outp", [C, T], f32, kind="ExternalOutput").ap()

    NCH = 4       # 512-wide t-chunks
    NJ = 16       # 128-wide s-tiles
    CHW = 512

    with tile.TileContext(nc) as tc, ExitStack() as ctx:
        sing = ctx.enter_context(tc.tile_pool(name="sing", bufs=1))
        xpool = ctx.enter_context(tc.tile_pool(name="xpool", bufs=2))
        ptp = ctx.enter_context(tc.tile_pool(name="ptp", bufs=3))
        rsp = ctx.enter_context(tc.tile_pool(name="rsp", bufs=2))
        rbp = ctx.enter_context(tc.tile_pool(name="rbp", bufs=2))
        osp = ctx.enter_context(tc.tile_pool(name="osp", bufs=3))
        ps = ctx.enter_context(tc.tile_pool(name="ps", bufs=2, space="PSUM"))

        # ---- resident SBUF tensors ----
        wk = sing.tile([128, 8, 768], f32r, name="wk")
        # q (slots 0,1) and vT (slots 2,3) by head-pair
        kqv = sing.tile([128, 4, T], f32r, name="kqv")
        # zero-padded kT for full-K QK matmuls: even heads in rows 0:64 of
        # ktp_e (rows 64:128 zero), odd heads in rows 64:128 of ktp_o
        ktp_e = sing.tile([128, 2, T], f32r, name="ktp_e")
        ktp_o = sing.tile([128, 2, T], f32r, name="ktp_o")
        # vsb: 32 blocks of [v_A(64) | ones(128) | v_B(64)]
        vsb = sing.tile([128, 32 * 256], f32r, name="vsb")
        ysb = sing.tile([128, 2, T], f32r, name="ysb")
        wp = sing.tile([128, 2, C], f32r, name="wp")
        bkq_s = sing.tile([128, 6], f32, name="bkq_s")
        bp_s = sing.tile([128, 8], f32, name="bp_s")
        ident = sing.tile([128, 128], f32, name="ident")
        amask = sing.tile([128, 128], f32, name="amask")

        nc.vector.memset(ktp_e[64:128, :, :].bitcast(f32), 0.0)
        nc.vector.memset(ktp_o[0:64, :, :].bitcast(f32), 0.0)

        xt_r = xt.rearrange("(kk p) t -> p kk t", p=128)
        wkqv_r = wkqv.rearrange("(kk p) f -> p kk f", p=128)
        for k in range(8):
            nc.sync.dma_start(wk[:, k, :], wkqv_r[:, k, :])
        nc.sync.dma_start(wp, wproj.rearrange("(kk p) f -> p kk f", p=128))
        nc.sync.dma_start(bkq_s, bkq)
        nc.sync.dma_start(bp_s, bp)
        nc.sync.dma_start(ident, ident_d)
        nc.sync.dma_start(amask, amask_d)
        vanchor = vsb[:, 64:65]
        vmid_view = bass.AP(tensor=vanchor.tensor, offset=vanchor.offset,
                            ap=[vanchor.ap[0], [256, 32], [1, 128]])
        nc.sync.dma_start(vmid_view,
                          vmid_d.rearrange("p (a b) -> p a b", a=32))

        # ---- phase 1: QKV projection ----
        # m: 0,1 -> kT (head pairs), 2,3 -> q, 4,5 -> v
        with nc.named_scope("qkv"):
            for n in range(NCH):
                xts = xpool.tile([128, 8, CHW], f32r, name="xts")
                for k in range(8):
                    nc.sync.dma_start(
                        xts[:, k, :],
                        xt_r[:, k, n * CHW:(n + 1) * CHW])
                for m in range(6):
                    acc = ps.tile([128, CHW], f32, name="acc", tag="acc")
                    for k in range(8):
                        nc.tensor.matmul(
                            acc, wk[:, k, m * 128:(m + 1) * 128], xts[:, k, :],
                            start=(k == 0), stop=(k == 7))
                    cols = slice(n * CHW, (n + 1) * CHW)
                    if m < 2:  # kT: split into zero-padded buffers
                        nc.vector.tensor_scalar_add(
                            out=ktp_e[0:64, m, cols], in0=acc[0:64, :],
                            scalar1=bkq_s[0:64, m:m + 1])
                        nc.vector.tensor_scalar_add(
                            out=ktp_o[64:128, m, cols], in0=acc[64:128, :],
                            scalar1=bkq_s[64:128, m:m + 1])
                    else:
                        nc.vector.tensor_scalar_add(
                            out=kqv[:, m - 2, cols], in0=acc,
                            scalar1=bkq_s[:, m:m + 1])

        # ---- phase 2: v transposes into vsb ----
        with nc.named_scope("vprep"):
            for j in range(NJ):
                for hf in range(2):
                    pt_ps = ps.tile([128, 128], f32, name="pt_ps", tag="aux")
                    nc.tensor.transpose(
                        pt_ps,
                        kqv[:, 2 + hf, j * 128:(j + 1) * 128].bitcast(f32),
                        ident)
                    off = (j * 2 + hf) * 256
                    anch = vsb[:, off:off + 1]
                    dst = bass.AP(tensor=anch.tensor, offset=anch.offset,
                                  ap=[anch.ap[0], [192, 2], [1, 64]])
                    nc.vector.tensor_copy(
                        dst, pt_ps.rearrange("p (a b) -> p a b", a=2))

        def v_stationary(j, h):
            """[128,128] AP: even slot -> [v_A|ones64], odd -> [ones64|v_B]."""
            hf, sl = h // 2, h % 2
            off = (j * 2 + hf) * 256 + 128 * sl
            return vsb[:, off:off + 128]

        # ---- phase 3: attention per head ----
        with nc.named_scope("attn"):
            for h in range(HPC):
                hf, sl = h // 2, h % 2
                ktp = ktp_e if sl == 0 else ktp_o
                for n in range(NCH):
                    jmax = 4 * n + 3
                    py = ps.tile([128, CHW], f32, name="py", tag="acc")
                    pend = []
                    for a in range(2 * n + 2):
                        ss = ps.tile([128, 2, CHW], f32, name="ss", tag="ss")
                        c0s = []
                        for idx in range(2):
                            j = 2 * a + idx
                            c0 = max(0, 128 * j - 512 * n)
                            c0s.append(c0)
                            nc.tensor.matmul(
                                ss[:, idx, c0:],
                                ktp[:, hf, j * 128:(j + 1) * 128],
                                kqv[:, hf, n * CHW + c0:(n + 1) * CHW],
                                start=True, stop=True)
                        for fn in pend:
                            fn()
                        pend = []
                        for idx in range(2):
                            j = 2 * a + idx
                            c0 = c0s[idx]
                            if j >= 4 * n:  # diagonal: additive causal mask
                                nc.vector.tensor_tensor(
                                    out=ss[:, idx, c0:c0 + 128],
                                    in0=ss[:, idx, c0:c0 + 128],
                                    in1=amask, op=OP.add)
                        pt = ptp.tile([128, 2, CHW], f32r, name="pt")
                        if c0s[0] == 0 and c0s[1] == 0:
                            nc.scalar.activation(out=pt, in_=ss, func=AF.Exp)
                        else:
                            for idx in range(2):
                                c0 = c0s[idx]
                                nc.scalar.activation(
                                    out=pt[:, idx, c0:], in_=ss[:, idx, c0:],
                                    func=AF.Exp)

                        def mk_pv(a=a, c0s=c0s, pt=pt):
                            for idx in range(2):
                                j = 2 * a + idx
                                c0 = c0s[idx]
                                nc.tensor.matmul(
                                    py[:, c0:], v_stationary(j, h),
                                    pt[:, idx, c0:],
                                    start=(j == 0), stop=(j == jmax),
                                    skip_group_check=True)
                        pend = [mk_pv]
                    for fn in pend:
                        fn()
                    # normalize: rowsum arrives pre-broadcast on the
                    # complement partitions; recip there, DMA-shift, multiply
                    ob = 64 * sl       # yT partitions
                    cb = 64 - ob       # rowsum-broadcast partitions
                    rbt = rsp.tile([128, CHW], f32, name="rbt")
                    nc.vector.reciprocal(rbt[cb:cb + 64, :],
                                         py[cb:cb + 64, :])
                    rb = rbp.tile([128, CHW], f32, name="rb")
                    nc.sync.dma_start(rb[ob:ob + 64, :], rbt[cb:cb + 64, :])
                    nc.vector.tensor_tensor(
                        out=ysb[ob:ob + 64, hf, n * CHW:(n + 1) * CHW],
                        in0=py[ob:ob + 64, :], in1=rb[ob:ob + 64, :],
                        op=OP.mult)

        # ---- phase 4: proj partial ----
        with nc.named_scope("proj"):
            for o in range(8):
                for n in range(NCH):
                    acc = ps.tile([128, CHW], f32, name="acc2", tag="acc")
                    for kk in range(2):
                        nc.tensor.matmul(
                            acc, wp[:, kk, o * 128:(o + 1) * 128],
                            ysb[:, kk, n * CHW:(n + 1) * CHW],
                            start=(kk == 0), stop=(kk == 1))
                    ot = osp.tile([128, CHW], f32, name="ot")
                    nc.scalar.activation(out=ot, in_=acc, func=AF.Identity,
                                         bias=bp_s[:, o:o + 1])
                    nc.sync.dma_start(
                        outp[o * 128:(o + 1) * 128, n * CHW:(n + 1) * CHW], ot)

    nc.compile()
    return nc


def _host_inputs(x, W_kqv, b_kqv, W_proj, b_proj):
    x = np.ascontiguousarray(np.asarray(x, dtype=np.float32))
    W_kqv = np.asarray(W_kqv, dtype=np.float32)
    b_kqv = np.asarray(b_kqv, dtype=np.float32)
    W_proj = np.asarray(W_proj, dtype=np.float32)
    b_proj = np.asarray(b_proj, dtype=np.float32)

    ident = np.eye(128, dtype=np.float32)
    ss, tt = np.meshgrid(np.arange(128), np.arange(128), indexing="ij")
    amask = np.where(ss <= tt, 0.0, NEG).astype(np.float32)
    # all-ones mid columns: PV matmul emits rowsum broadcast over the
    # complement 64 partitions of each head's yT
    vmid = np.ones((128, 4096), dtype=np.float32)

    xts = [np.ascontiguousarray(x[b].T) for b in range(B)]

    in_maps = []
    for c in range(NCORES):
        b, g = c // 4, c % 4
        heads = [4 * g + i for i in range(HPC)]
        wl = np.concatenate(
            [W_kqv[h * 192:h * 192 + 64] for h in heads]
            + [W_kqv[h * 192 + 64:h * 192 + 128] * 0.125 for h in heads]
            + [W_kqv[h * 192 + 128:h * 192 + 192] for h in heads], axis=0)
        bl = np.concatenate(
            [b_kqv[h * 192:h * 192 + 64] for h in heads]
            + [b_kqv[h * 192 + 64:h * 192 + 128] * 0.125 for h in heads]
            + [b_kqv[h * 192 + 128:h * 192 + 192] for h in heads])
        bpl = b_proj if g == 0 else np.zeros_like(b_proj)
        in_maps.append({
            "xt": xts[b],
            "wkqv": np.ascontiguousarray(wl.T),
            "bkq": np.ascontiguousarray(bl.reshape(6, 128).T),
            "wproj": np.ascontiguousarray(
                W_proj[:, 256 * g:256 * (g + 1)].T),
            "bp": np.ascontiguousarray(bpl.reshape(8, 128).T),
            "ident": ident,
            "amask": amask,
            "vmid": vmid,
        })
    return in_maps


def kernel(x, W_kqv, b_kqv, W_proj, b_proj):
    from concourse.bass_utils import run_bass_kernel_spmd

    if "nc" not in _cache:
        _cache["nc"] = _build_nc()
    nc = _cache["nc"]

    in_maps = _host_inputs(x, W_kqv, b_kqv, W_proj, b_proj)
    trace = bool(int(os.environ.get("KERNEL_TRACE", "0")))
    r = run_bass_kernel_spmd(nc, in_maps, core_ids=list(range(NCORES)),
                             trace=trace)
    if trace:
        _cache["last_results"] = r
        print(f"HW exec time: {r.exec_time_ns} ns")

    out = np.empty((B, T, C), dtype=np.float32)
    for b in range(B):
        acc = np.zeros((C, T), dtype=np.float32)
        for g in range(4):
            acc += r.results[4 * b + g]["outp"]
        out[b] = acc.T
    return out
